# revision 37
# baseline (speedup 1.0000x reference)
"""Trainium2 Bass kernel: 16-head attention (SEQ=4096, D_MODEL=1024, D_K=64).

Sharding: tensor-parallel over heads. 2 heads per core x 8 cores.
W_O is row-sharded; each core returns a partial [S, D] output projection,
summed on the host (the all-reduce of the output projection).

Per-core dataflow (all matmuls fp32r = FP22-truncated full-rate):
  qT/kT [64,S] per head via projections on transposed inputs (QT/KT in DRAM)
  v natural [S,64] per head (direct matmul from VT chunks), augmented with a
    ones column so attention@V also yields softmax row-sums.
  Pass 1 (natural scores [q,s]): row maxes via DVE reduce_max.
  maxes transposed via a tiny matmul against -I, written into row 64 of the
    augmented qT tiles.
  Pass 2 (transposed scores [s,q]): K=65 matmul computes scores^T - max(q)
    directly; ScalarE exp -> E; [v|1] @ E accumulates att@V and row sums.
  Normalize via reciprocal + ones-replication matmul; W_O on normalized
    concatenated heads; partial out DMA'd back.

Scheduling: one global chunk pipeline keeps every engine dense so the PE HAM
clock-gate stays warm: DMA submissions are split from the matmuls that
consume them, the natural (max) pass for tile t+1 and the q projection for
tile t+2 stream inside tile t, and tile t-1's normalize/W_O epilogue is
spread over tile t's early chunks via an SBUF bounce of the av accumulators.
"""

import os
import sys

import numpy as np

for _p in (
    "/root/.axon_site",
    "/root/.axon_site/_ro/trn_rl_repo",
    "/root/.axon_site/_ro/pypackages",
    "/opt/trn_rl_repo",
    "/opt/pypackages",
):
    if os.path.isdir(_p) and _p not in sys.path:
        sys.path.append(_p)

D = 1024
NHEADS = 16
DK = 64
NCORES = 8
S_FULL = 4096

_cache = {}
LAST_RESULT = None  # BassKernelResults of the most recent run (for test harness)


def _build(S):
    import concourse.bass as bass  # noqa: F401
    import concourse.tile as tile
    from concourse import bacc, mybir
    from concourse.masks import make_identity
    from contextlib import ExitStack

    f32 = mybir.dt.float32
    f32r = mybir.dt.float32r
    fp16 = mybir.dt.float16
    X = mybir.AxisListType.X
    Exp = mybir.ActivationFunctionType.Exp

    NT = S // 512   # 512-wide q tiles
    NCH = S // 128  # 128-wide s chunks per tile
    ND = D // 128   # contraction chunks

    nc = bacc.Bacc(
        "TRN2",
        target_bir_lowering=False,
        debug=False,
        num_devices=NCORES,
    )
    qt = nc.dram_tensor("qt", [D, S], f32r, kind="ExternalInput")
    kt = nc.dram_tensor("kt", [D, S], f32r, kind="ExternalInput")
    vt = nc.dram_tensor("vt", [D, S], fp16, kind="ExternalInput")
    wq = nc.dram_tensor("wq", [D, 128], f32r, kind="ExternalInput")
    wk = nc.dram_tensor("wk", [D, 128], f32r, kind="ExternalInput")
    wv = nc.dram_tensor("wv", [D, 128], fp16, kind="ExternalInput")
    wo = nc.dram_tensor("wo", [128, D], f32r, kind="ExternalInput")
    out = nc.dram_tensor("out", [S, D], f32, kind="ExternalOutput")
    dbg = None
    if os.environ.get("KDEBUG"):
        dbg = {
            "d_qaug0": nc.dram_tensor("d_qaug0", [65, S], f32, kind="ExternalOutput"),
            "d_qaug1": nc.dram_tensor("d_qaug1", [65, S], f32, kind="ExternalOutput"),
            "d_concat": nc.dram_tensor("d_concat", [128, S], f32, kind="ExternalOutput"),
            "d_v": nc.dram_tensor("d_v", [128, S // 128 * 2 * 65],
                                  mybir.dt.float16, kind="ExternalOutput"),
        }

    with tile.TileContext(nc) as tc, ExitStack() as ctx:
        consts = ctx.enter_context(tc.tile_pool(name="consts", bufs=1))
        big = ctx.enter_context(tc.tile_pool(name="big", bufs=1))
        ldpool = ctx.enter_context(tc.tile_pool(name="ld", bufs=2))
        epool = ctx.enter_context(tc.tile_pool(name="e", bufs=12))
        smalls = ctx.enter_context(tc.tile_pool(name="smalls", bufs=4))
        outp = ctx.enter_context(tc.tile_pool(name="outp", bufs=2))
        # PSUM budget (8 banks): pb [128,1024] x2 bufs = 4, pf [128,512] x2 = 2,
        # pav0/pav1 [65,512] = 2. Double-buffered pb decouples the nat-pass
        # matmuls from the DVE reduce_max of the previous unit.
        ps_big = ctx.enter_context(tc.tile_pool(name="ps_big", bufs=2, space="PSUM"))
        ps_av = ctx.enter_context(tc.tile_pool(name="ps_av", bufs=1, space="PSUM"))
        ps_f = ctx.enter_context(tc.tile_pool(name="ps_f", bufs=2, space="PSUM"))

        def pbig():
            return ps_big.tile([128, 1024], f32, tag="pb", name="pb")

        def pfine():
            return ps_f.tile([128, 512], f32, tag="pf", name="pf")

        # constants
        ident_f = consts.tile([128, 128], f32)
        make_identity(nc, ident_f)
        identn = consts.tile([128, 128], f32r)  # -I, rounded for fp32r matmul
        nc.vector.tensor_scalar_mul(identn[:], ident_f[:], -1.0)
        ones64 = consts.tile([1, 64], f32r)
        nc.vector.memset(ones64[:].bitcast(f32), 1.0)

        # weights
        wq_sb = consts.tile([128, ND, 128], f32r)
        nc.sync.dma_start(wq_sb[:], wq.rearrange("(o p) f -> p o f", p=128))
        wk_sb = consts.tile([128, ND, 128], f32r)
        nc.sync.dma_start(wk_sb[:], wk.rearrange("(o p) f -> p o f", p=128))
        wv_sb = consts.tile([128, ND, 128], fp16)
        nc.sync.dma_start(wv_sb[:], wv.rearrange("(o p) f -> p o f", p=128))
        wo_sb = consts.tile([128, D], f32r)
        nc.sync.dma_start(wo_sb[:], wo[:])

        # big SBUF tensors
        qaug = [big.tile([65, S], f32r, tag=f"qaug{h}", name=f"qaug{h}") for h in range(2)]
        kaug = [big.tile([65, S], f32r, tag=f"kaug{h}", name=f"kaug{h}") for h in range(2)]
        v_sb = big.tile([128, NCH, 2, 65], fp16, tag="v", name="v_sb")
        q16 = big.tile([128, S], fp16, tag="q16", name="q16")
        k16 = big.tile([128, S], fp16, tag="k16", name="k16")
        concat = big.tile([128, S], f32r, tag="concat", name="concat")
        for h in range(2):
            nc.vector.memset(kaug[h][64:65, :].bitcast(f32), 1.0)
        nc.vector.memset(v_sb[:, :, :, 64:65], 1.0)

        # ---- projection pieces (DMA submission split from the matmuls)
        def proj_load(src, t, dt=f32r):
            lt = ldpool.tile([128, ND, 512], dt, tag="ld", name="ld")
            nc.sync.dma_start(
                lt[:], src[:, t * 512:(t + 1) * 512].rearrange("(o p) s -> p o s", p=128))
            return lt

        def proj_mms(lt, wsb, t, dstA, dstB, dst16):
            ps = pbig()[:, :512]
            for d in range(ND):
                nc.tensor.matmul(ps, wsb[:, d, :], lt[:, d, :],
                                 start=(d == 0), stop=(d == ND - 1))
            tsl = slice(t * 512, (t + 1) * 512)
            nc.scalar.copy(dstA[0:64, tsl], ps[0:64, :])
            nc.scalar.copy(dstB[0:64, tsl], ps[64:128, :])
            nc.vector.tensor_copy(dst16[:, tsl], ps[:])

        def v_mms(vts, t):
            ps = pbig()[:, :512]
            for sc in range(4):
                for d in range(ND):
                    nc.tensor.matmul(ps[:, sc * 128:(sc + 1) * 128],
                                     vts[:, d, sc * 128:(sc + 1) * 128],
                                     wv_sb[:, d, :],
                                     start=(d == 0), stop=(d == ND - 1),
                                     skip_group_check=True)
            for sc in range(4):
                c = t * 4 + sc
                nc.vector.tensor_copy(
                    v_sb[:, c, :, 0:64],
                    ps[:, sc * 128:(sc + 1) * 128].rearrange("p (h f) -> p h f", h=2))

        # ---- natural (max) pass units
        mxs_all = {}

        def nat_unit(b, sh):
            qsl = slice(b * 128, (b + 1) * 128)
            psn = pbig()
            ssl = slice(sh * 512, (sh + 1) * 512)
            # the two K=64 head matmuls run concurrently in the PE array via
            # 64x128 row tiling: head h reads SBUF partitions [64h, 64h+64)
            for h in range(2):
                hp = slice(h * 64, h * 64 + 64)
                nc.tensor.matmul(psn[:, h * 512:(h + 1) * 512],
                                 q16[hp, qsl], k16[hp, ssl],
                                 start=True, stop=True,
                                 tile_position=(h * 64, 0))
            for h in range(2):
                if sh == 0:
                    mxs_all[(b, h)] = smalls.tile(
                        [128, S // 512], f32, tag=f"mx{h}", name=f"mx{h}")
                nc.vector.reduce_max(mxs_all[(b, h)][:, sh:sh + 1],
                                     psn[:, h * 512:(h + 1) * 512], axis=X)

        def nat_finish(b):
            qsl = slice(b * 128, (b + 1) * 128)
            m2 = smalls.tile([128, 2], f32r, tag="m2", name="m2")
            for h in range(2):
                nc.vector.reduce_max(m2[:, h:h + 1], mxs_all.pop((b, h))[:], axis=X)
            psmt = pbig()
            for h in range(2):
                nc.tensor.matmul(psmt[0:1, h * 128:(h + 1) * 128],
                                 m2[:, h:h + 1], identn[:],
                                 start=True, stop=True)
                nc.vector.tensor_copy(qaug[h][64:65, qsl],
                                      psmt[0:1, h * 128:(h + 1) * 128])

        # ---- prologue: pipeline k/q loads with the tile-0 natural pass.
        # nat units for s-block sh only need k16[:, sh*512:...], so they
        # stream two blocks behind the k-projection matmuls.
        ltk = proj_load(kt, 0)
        ltq = proj_load(qt, 0)
        proj_mms(ltk, wk_sb, 0, kaug[0], kaug[1], k16)
        pending = proj_load(kt, 1)
        proj_mms(ltq, wq_sb, 0, qaug[0], qaug[1], q16)
        ltq1 = proj_load(qt, 1)
        proj_mms(pending, wk_sb, 1, kaug[0], kaug[1], k16)
        pending = proj_load(kt, 2)
        proj_mms(ltq1, wq_sb, 1, qaug[0], qaug[1], q16)
        # prologue-only paired nat: same-head sh pairs share one [128,1024]
        # tile so the max reduce runs FD=1024; safe here because no other pb
        # user is emitted between a pair's allocation and its reduces
        def nat_pair0(b, shp):
            qsl = slice(b * 128, (b + 1) * 128)
            pA = pbig()
            pB = pbig()
            for k_, sh in enumerate((2 * shp, 2 * shp + 1)):
                ssl = slice(sh * 512, (sh + 1) * 512)
                half = slice(k_ * 512, k_ * 512 + 512)
                nc.tensor.matmul(pA[:, half], q16[0:64, qsl], k16[0:64, ssl],
                                 start=True, stop=True, tile_position=(0, 0))
                nc.tensor.matmul(pB[:, half], q16[64:128, qsl], k16[64:128, ssl],
                                 start=True, stop=True, tile_position=(64, 0))
            for h, pp in ((0, pA), (1, pB)):
                if shp == 0:
                    mxs_all[(b, h)] = smalls.tile(
                        [128, S // 1024], f32, tag=f"mx{h}", name=f"mx{h}")
                nc.vector.reduce_max(mxs_all[(b, h)][:, shp:shp + 1], pp[:], axis=X)

        for b in range(4):
            nat_pair0(b, 0)
        for j in range(2, NT):
            proj_mms(pending, wk_sb, j, kaug[0], kaug[1], k16)
            if j + 1 < NT:
                pending = proj_load(kt, j + 1)
            if j % 2 == 1:
                for b in range(4):
                    nat_pair0(b, (j - 1) // 2)
        for b in range(4):
            nat_finish(b)
        lv = {0: proj_load(vt, 0, fp16), 1: proj_load(vt, 1, fp16)}

        # ---- steady-state pipeline over tiles
        AVSKEW = 4
        es = {}
        psA_by_tile = {}

        def emit_fine(t, c):
            tsl = slice(t * 512, (t + 1) * 512)
            for h in range(2):
                psf = pfine()
                nc.tensor.matmul(psf[:],
                                 kaug[h][:, c * 128:(c + 1) * 128],
                                 qaug[h][:, tsl],
                                 start=True, stop=True)
                e = epool.tile([128, 512], fp16, tag="e", name="e")
                nc.scalar.activation(e[:], psf[:], Exp)
                es[(t, c, h)] = e

        def emit_av(t, c):
            if c == 0:
                psA_by_tile[t] = [
                    ps_av.tile([65, 512], f32, tag=f"pav{h}", name=f"pav{h}")
                    for h in range(2)]
            psA = psA_by_tile[t]
            for h in range(2):
                nc.tensor.matmul(psA[h][:], v_sb[:, c, h, :],
                                 es.pop((t, c, h))[:],
                                 start=(c == 0), stop=(c == NCH - 1),
                                 skip_group_check=True)

        def emit_normalize(t, h):
            tsl = slice(t * 512, (t + 1) * 512)
            psA_h = psA_by_tile[t][h]
            sums = smalls.tile([1, 512], f32, tag="sums", name="sums")
            nc.vector.tensor_copy(sums[:], psA_h[64:65, :])
            rec = smalls.tile([1, 512], f32, tag="rec", name="rec")
            nc.vector.reciprocal_approx_fast(rec[:], sums[:])
            rec_r = smalls.tile([1, 512], f32r, tag="rec_r", name="rec_r")
            nc.vector.tensor_copy(rec_r[:], rec[:])
            psr = pfine()
            nc.tensor.matmul(psr[0:64, :], ones64[:], rec_r[:],
                             start=True, stop=True)
            reps = smalls.tile([64, 512], f32, tag="reps", name="reps")
            nc.scalar.copy(reps[:], psr[0:64, :])
            nc.vector.tensor_mul(concat[h * 64:(h + 1) * 64, tsl],
                                 psA_h[0:64, :], reps[:])

        def emit_wo(t, b):
            qb = t * 4 + b
            pso = pbig()
            for n in range(2):
                nc.tensor.matmul(pso[:, n * 512:(n + 1) * 512],
                                 concat[:, qb * 128:(qb + 1) * 128],
                                 wo_sb[:, n * 512:(n + 1) * 512],
                                 start=True, stop=True)
            ot = outp.tile([128, 1024], f32, tag="ot", name="ot")
            if b % 2 == 0:
                nc.scalar.copy(ot[:], pso[:])
            else:
                nc.vector.tensor_copy(ot[:], pso[:])
            nc.sync.dma_start(out[qb * 128:(qb + 1) * 128, :], ot[:])

        qp_pending = {}
        for t in range(NT):
            for c in range(NCH):
                emit_fine(t, c)
                g = (t * NCH + c) - AVSKEW
                if g >= 0:
                    ta, ca = divmod(g, NCH)
                    emit_av(ta, ca)
                # stream next tile's natural pass, one unit per chunk; the
                # max write into qaug row 64 rides a DMA whose completion
                # semaphore orders it against the fine matmuls of tile t+1
                if t + 1 < NT:
                    bl, sh = divmod(c, S // 512)
                    if bl < 4:
                        nat_unit((t + 1) * 4 + bl, sh)
                    if c % (S // 512) == 0 and 1 <= bl <= 3:
                        nat_finish((t + 1) * 4 + bl - 1)
                    if c == NCH - 1:
                        nat_finish((t + 1) * 4 + 3)
                # v projection streams through tile 0 at the DMA cadence
                if t == 0 and c >= 3 and (c - 3) % 3 == 0:
                    j = (c - 3) // 3
                    if j < NT:
                        v_mms(lv.pop(j), j)
                        if j + 2 < NT:
                            lv[j + 2] = proj_load(vt, j + 2, fp16)
                # epilogue of tile t-1 (avs for t-1 finish at c==3); both
                # normalizes emit at c==3 so all psA readers precede the
                # pav ring reallocation at c==4
                if t > 0:
                    if c == 3:
                        emit_normalize(t - 1, 0)
                        emit_normalize(t - 1, 1)
                    elif 6 <= c < 10:
                        emit_wo(t - 1, c - 6)
                # q projection for tile t+2: DMA early, matmuls later
                if t + 2 < NT:
                    if c == (27 if t == 0 else 2):
                        qp_pending[t + 2] = proj_load(qt, t + 2)
                    elif c == (30 if t == 0 else 16):
                        proj_mms(qp_pending.pop(t + 2), wq_sb, t + 2,
                                 qaug[0], qaug[1], q16)

        # drain: remaining avs, then last tile's epilogue
        for g in range(NT * NCH - AVSKEW, NT * NCH):
            ta, ca = divmod(g, NCH)
            emit_av(ta, ca)
        emit_normalize(NT - 1, 0)
        emit_normalize(NT - 1, 1)
        for b in range(4):
            emit_wo(NT - 1, b)

        if dbg is not None:
            nc.sync.dma_start(dbg["d_qaug0"][:], qaug[0][:].bitcast(f32))
            nc.sync.dma_start(dbg["d_qaug1"][:], qaug[1][:].bitcast(f32))
            nc.sync.dma_start(dbg["d_concat"][:], concat[:].bitcast(f32))
            nc.sync.dma_start(dbg["d_v"][:], v_sb[:].rearrange("p a b c -> p (a b c)"))

    nc.compile()
    return nc


def _prep_inputs(Q, K, V, W_Q, W_K, W_V, W_O):
    Q = np.ascontiguousarray(np.asarray(Q, dtype=np.float32))
    K = np.ascontiguousarray(np.asarray(K, dtype=np.float32))
    V = np.ascontiguousarray(np.asarray(V, dtype=np.float32))
    W_Q = np.asarray(W_Q, dtype=np.float32)
    W_K = np.asarray(W_K, dtype=np.float32)
    W_V = np.asarray(W_V, dtype=np.float32)
    W_O = np.asarray(W_O, dtype=np.float32)

    QT = np.ascontiguousarray(Q.T)
    KT = np.ascontiguousarray(K.T)
    VT = np.ascontiguousarray(V.T.astype(np.float16))
    scale = np.float32(0.125)  # 1/sqrt(64), exact power of two

    in_maps = []
    for c in range(NCORES):
        hA, hB = 2 * c, 2 * c + 1
        in_maps.append({
            "qt": QT,
            "kt": KT,
            "vt": VT,
            "wq": np.ascontiguousarray(np.concatenate([W_Q[hA], W_Q[hB]], axis=1)),
            "wk": np.ascontiguousarray(
                np.concatenate([W_K[hA] * scale, W_K[hB] * scale], axis=1)),
            "wv": np.ascontiguousarray(
                np.concatenate([W_V[hA], W_V[hB]], axis=1).astype(np.float16)),
            "wo": np.ascontiguousarray(W_O[c * 128:(c + 1) * 128, :]),
        })
    return in_maps


def kernel(Q, K, V, W_Q, W_K, W_V, W_O):
    global LAST_RESULT
    from concourse.bass_utils import run_bass_kernel_spmd

    S = np.asarray(Q).shape[0]
    nc = _cache.get(S)
    if nc is None:
        nc = _build(S)
        _cache[S] = nc

    in_maps = _prep_inputs(Q, K, V, W_Q, W_K, W_V, W_O)
    res = run_bass_kernel_spmd(nc, in_maps, list(range(NCORES)))
    LAST_RESULT = res
    parts = np.stack([res.results[i]["out"] for i in range(NCORES)])
    return parts.sum(axis=0, dtype=np.float32)


# revision 39
# speedup vs baseline: 1.0035x; 1.0035x over previous
"""Trainium2 Bass kernel: 16-head attention (SEQ=4096, D_MODEL=1024, D_K=64).

Sharding: tensor-parallel over heads. 2 heads per core x 8 cores.
W_O is row-sharded; each core returns a partial [S, D] output projection,
summed on the host (the all-reduce of the output projection).

Per-core dataflow (all matmuls fp32r = FP22-truncated full-rate):
  qT/kT [64,S] per head via projections on transposed inputs (QT/KT in DRAM)
  v natural [S,64] per head (direct matmul from VT chunks), augmented with a
    ones column so attention@V also yields softmax row-sums.
  Pass 1 (natural scores [q,s]): row maxes via DVE reduce_max.
  maxes transposed via a tiny matmul against -I, written into row 64 of the
    augmented qT tiles.
  Pass 2 (transposed scores [s,q]): K=65 matmul computes scores^T - max(q)
    directly; ScalarE exp -> E; [v|1] @ E accumulates att@V and row sums.
  Normalize via reciprocal + ones-replication matmul; W_O on normalized
    concatenated heads; partial out DMA'd back.

Scheduling: one global chunk pipeline keeps every engine dense so the PE HAM
clock-gate stays warm: DMA submissions are split from the matmuls that
consume them, the natural (max) pass for tile t+1 and the q projection for
tile t+2 stream inside tile t, and tile t-1's normalize/W_O epilogue is
spread over tile t's early chunks via an SBUF bounce of the av accumulators.
"""

import os
import sys

import numpy as np

for _p in (
    "/root/.axon_site",
    "/root/.axon_site/_ro/trn_rl_repo",
    "/root/.axon_site/_ro/pypackages",
    "/opt/trn_rl_repo",
    "/opt/pypackages",
):
    if os.path.isdir(_p) and _p not in sys.path:
        sys.path.append(_p)

D = 1024
NHEADS = 16
DK = 64
NCORES = 8
S_FULL = 4096

_cache = {}
LAST_RESULT = None  # BassKernelResults of the most recent run (for test harness)


def _build(S):
    import concourse.bass as bass  # noqa: F401
    import concourse.tile as tile
    from concourse import bacc, mybir
    from concourse.masks import make_identity
    from contextlib import ExitStack

    f32 = mybir.dt.float32
    f32r = mybir.dt.float32r
    fp16 = mybir.dt.float16
    X = mybir.AxisListType.X
    Exp = mybir.ActivationFunctionType.Exp

    NT = S // 512   # 512-wide q tiles
    NCH = S // 128  # 128-wide s chunks per tile
    ND = D // 128   # contraction chunks

    nc = bacc.Bacc(
        "TRN2",
        target_bir_lowering=False,
        debug=False,
        num_devices=NCORES,
    )
    qt = nc.dram_tensor("qt", [D, S], f32r, kind="ExternalInput")
    kt = nc.dram_tensor("kt", [D, S], f32r, kind="ExternalInput")
    vt = nc.dram_tensor("vt", [D, S], fp16, kind="ExternalInput")
    wq = nc.dram_tensor("wq", [D, 128], f32r, kind="ExternalInput")
    wk = nc.dram_tensor("wk", [D, 128], f32r, kind="ExternalInput")
    wv = nc.dram_tensor("wv", [D, 128], fp16, kind="ExternalInput")
    wo = nc.dram_tensor("wo", [128, D], fp16, kind="ExternalInput")
    out = nc.dram_tensor("out", [S, D], fp16, kind="ExternalOutput")
    dbg = None
    if os.environ.get("KDEBUG"):
        dbg = {
            "d_qaug0": nc.dram_tensor("d_qaug0", [65, S], f32, kind="ExternalOutput"),
            "d_qaug1": nc.dram_tensor("d_qaug1", [65, S], f32, kind="ExternalOutput"),
            "d_concat": nc.dram_tensor("d_concat", [128, S], f32, kind="ExternalOutput"),
            "d_v": nc.dram_tensor("d_v", [128, S // 128 * 2 * 65],
                                  mybir.dt.float16, kind="ExternalOutput"),
        }

    with tile.TileContext(nc) as tc, ExitStack() as ctx:
        consts = ctx.enter_context(tc.tile_pool(name="consts", bufs=1))
        big = ctx.enter_context(tc.tile_pool(name="big", bufs=1))
        ldpool = ctx.enter_context(tc.tile_pool(name="ld", bufs=2))
        epool = ctx.enter_context(tc.tile_pool(name="e", bufs=12))
        smalls = ctx.enter_context(tc.tile_pool(name="smalls", bufs=4))
        outp = ctx.enter_context(tc.tile_pool(name="outp", bufs=2))
        # PSUM budget (8 banks): pb [128,1024] x2 bufs = 4, pf [128,512] x2 = 2,
        # pav0/pav1 [65,512] = 2. Double-buffered pb decouples the nat-pass
        # matmuls from the DVE reduce_max of the previous unit.
        ps_big = ctx.enter_context(tc.tile_pool(name="ps_big", bufs=2, space="PSUM"))
        ps_av = ctx.enter_context(tc.tile_pool(name="ps_av", bufs=1, space="PSUM"))
        ps_f = ctx.enter_context(tc.tile_pool(name="ps_f", bufs=2, space="PSUM"))

        def pbig():
            return ps_big.tile([128, 1024], f32, tag="pb", name="pb")

        def pfine():
            return ps_f.tile([128, 512], f32, tag="pf", name="pf")

        # constants
        ident_f = consts.tile([128, 128], f32)
        make_identity(nc, ident_f)
        identn = consts.tile([128, 128], f32r)  # -I, rounded for fp32r matmul
        nc.vector.tensor_scalar_mul(identn[:], ident_f[:], -1.0)
        ones64 = consts.tile([1, 64], f32r)
        nc.vector.memset(ones64[:].bitcast(f32), 1.0)

        # weights
        wq_sb = consts.tile([128, ND, 128], f32r)
        nc.sync.dma_start(wq_sb[:], wq.rearrange("(o p) f -> p o f", p=128))
        wk_sb = consts.tile([128, ND, 128], f32r)
        nc.sync.dma_start(wk_sb[:], wk.rearrange("(o p) f -> p o f", p=128))
        wv_sb = consts.tile([128, ND, 128], fp16)
        nc.sync.dma_start(wv_sb[:], wv.rearrange("(o p) f -> p o f", p=128))
        wo_sb = consts.tile([128, D], fp16)
        nc.sync.dma_start(wo_sb[:], wo[:])

        # big SBUF tensors
        qaug = [big.tile([65, S], f32r, tag=f"qaug{h}", name=f"qaug{h}") for h in range(2)]
        kaug = [big.tile([65, S], f32r, tag=f"kaug{h}", name=f"kaug{h}") for h in range(2)]
        v_sb = big.tile([128, NCH, 2, 65], fp16, tag="v", name="v_sb")
        q16 = big.tile([128, S], fp16, tag="q16", name="q16")
        k16 = big.tile([128, S], fp16, tag="k16", name="k16")
        concat = big.tile([128, S], fp16, tag="concat", name="concat")
        for h in range(2):
            nc.vector.memset(kaug[h][64:65, :].bitcast(f32), 1.0)
        nc.vector.memset(v_sb[:, :, :, 64:65], 1.0)

        # ---- projection pieces (DMA submission split from the matmuls)
        def proj_load(src, t, dt=f32r):
            lt = ldpool.tile([128, ND, 512], dt, tag="ld", name="ld")
            nc.sync.dma_start(
                lt[:], src[:, t * 512:(t + 1) * 512].rearrange("(o p) s -> p o s", p=128))
            return lt

        def proj_mms(lt, wsb, t, dstA, dstB, dst16):
            ps = pbig()[:, :512]
            for d in range(ND):
                nc.tensor.matmul(ps, wsb[:, d, :], lt[:, d, :],
                                 start=(d == 0), stop=(d == ND - 1))
            tsl = slice(t * 512, (t + 1) * 512)
            nc.scalar.copy(dstA[0:64, tsl], ps[0:64, :])
            nc.scalar.copy(dstB[0:64, tsl], ps[64:128, :])
            nc.vector.tensor_copy(dst16[:, tsl], ps[:])

        def v_mms(vts, t):
            ps = pbig()[:, :512]
            for sc in range(4):
                for d in range(ND):
                    nc.tensor.matmul(ps[:, sc * 128:(sc + 1) * 128],
                                     vts[:, d, sc * 128:(sc + 1) * 128],
                                     wv_sb[:, d, :],
                                     start=(d == 0), stop=(d == ND - 1),
                                     skip_group_check=True)
            for sc in range(4):
                c = t * 4 + sc
                nc.vector.tensor_copy(
                    v_sb[:, c, :, 0:64],
                    ps[:, sc * 128:(sc + 1) * 128].rearrange("p (h f) -> p h f", h=2))

        # ---- natural (max) pass units
        mxs_all = {}

        def nat_unit(b, sh):
            qsl = slice(b * 128, (b + 1) * 128)
            psn = pbig()
            ssl = slice(sh * 512, (sh + 1) * 512)
            # the two K=64 head matmuls run concurrently in the PE array via
            # 64x128 row tiling: head h reads SBUF partitions [64h, 64h+64)
            for h in range(2):
                hp = slice(h * 64, h * 64 + 64)
                nc.tensor.matmul(psn[:, h * 512:(h + 1) * 512],
                                 q16[hp, qsl], k16[hp, ssl],
                                 start=True, stop=True,
                                 tile_position=(h * 64, 0))
            for h in range(2):
                if sh == 0:
                    mxs_all[(b, h)] = smalls.tile(
                        [128, S // 512], f32, tag=f"mx{h}", name=f"mx{h}")
                nc.vector.reduce_max(mxs_all[(b, h)][:, sh:sh + 1],
                                     psn[:, h * 512:(h + 1) * 512], axis=X)

        def nat_finish(b):
            qsl = slice(b * 128, (b + 1) * 128)
            m2 = smalls.tile([128, 2], f32r, tag="m2", name="m2")
            for h in range(2):
                nc.vector.reduce_max(m2[:, h:h + 1], mxs_all.pop((b, h))[:], axis=X)
            psmt = pbig()
            for h in range(2):
                nc.tensor.matmul(psmt[0:1, h * 128:(h + 1) * 128],
                                 m2[:, h:h + 1], identn[:],
                                 start=True, stop=True)
                nc.vector.tensor_copy(qaug[h][64:65, qsl],
                                      psmt[0:1, h * 128:(h + 1) * 128])

        # ---- prologue: pipeline k/q loads with the tile-0 natural pass.
        # nat units for s-block sh only need k16[:, sh*512:...], so they
        # stream two blocks behind the k-projection matmuls.
        ltk = proj_load(kt, 0)
        ltq = proj_load(qt, 0)
        proj_mms(ltk, wk_sb, 0, kaug[0], kaug[1], k16)
        pending = proj_load(kt, 1)
        proj_mms(ltq, wq_sb, 0, qaug[0], qaug[1], q16)
        ltq1 = proj_load(qt, 1)
        proj_mms(pending, wk_sb, 1, kaug[0], kaug[1], k16)
        pending = proj_load(kt, 2)
        proj_mms(ltq1, wq_sb, 1, qaug[0], qaug[1], q16)
        for j in range(2, NT):
            for b in range(4):
                nat_unit(b, j - 2)
            proj_mms(pending, wk_sb, j, kaug[0], kaug[1], k16)
            if j + 1 < NT:
                pending = proj_load(kt, j + 1)
        for sh in (NT - 2, NT - 1):
            for b in range(4):
                nat_unit(b, sh)
        for b in range(4):
            nat_finish(b)
        lv = {0: proj_load(vt, 0, fp16), 1: proj_load(vt, 1, fp16)}

        # ---- steady-state pipeline over tiles
        AVSKEW = 4
        es = {}
        psA_by_tile = {}

        def emit_fine(t, c):
            tsl = slice(t * 512, (t + 1) * 512)
            for h in range(2):
                psf = pfine()
                nc.tensor.matmul(psf[:],
                                 kaug[h][:, c * 128:(c + 1) * 128],
                                 qaug[h][:, tsl],
                                 start=True, stop=True)
                e = epool.tile([128, 512], fp16, tag="e", name="e")
                nc.scalar.activation(e[:], psf[:], Exp)
                es[(t, c, h)] = e

        def emit_av(t, c):
            if c == 0:
                psA_by_tile[t] = [
                    ps_av.tile([65, 512], f32, tag=f"pav{h}", name=f"pav{h}")
                    for h in range(2)]
            psA = psA_by_tile[t]
            for h in range(2):
                nc.tensor.matmul(psA[h][:], v_sb[:, c, h, :],
                                 es.pop((t, c, h))[:],
                                 start=(c == 0), stop=(c == NCH - 1),
                                 skip_group_check=True)

        def emit_normalize(t, h):
            tsl = slice(t * 512, (t + 1) * 512)
            psA_h = psA_by_tile[t][h]
            sums = smalls.tile([1, 512], f32, tag="sums", name="sums")
            nc.vector.tensor_copy(sums[:], psA_h[64:65, :])
            rec = smalls.tile([1, 512], f32, tag="rec", name="rec")
            nc.vector.reciprocal_approx_fast(rec[:], sums[:])
            rec_r = smalls.tile([1, 512], f32r, tag="rec_r", name="rec_r")
            nc.vector.tensor_copy(rec_r[:], rec[:])
            psr = pfine()
            nc.tensor.matmul(psr[0:64, :], ones64[:], rec_r[:],
                             start=True, stop=True)
            reps = smalls.tile([64, 512], f32, tag="reps", name="reps")
            nc.scalar.copy(reps[:], psr[0:64, :])
            nc.vector.tensor_mul(concat[h * 64:(h + 1) * 64, tsl],
                                 psA_h[0:64, :], reps[:])

        def emit_wo(t, b):
            qb = t * 4 + b
            pso = pbig()
            for n in range(2):
                nc.tensor.matmul(pso[:, n * 512:(n + 1) * 512],
                                 concat[:, qb * 128:(qb + 1) * 128],
                                 wo_sb[:, n * 512:(n + 1) * 512],
                                 start=True, stop=True)
            ot = outp.tile([128, 1024], fp16, tag="ot", name="ot")
            if b % 2 == 0:
                nc.scalar.copy(ot[:], pso[:])
            else:
                nc.vector.tensor_copy(ot[:], pso[:])
            nc.sync.dma_start(out[qb * 128:(qb + 1) * 128, :], ot[:])

        qp_pending = {}
        for t in range(NT):
            for c in range(NCH):
                emit_fine(t, c)
                g = (t * NCH + c) - AVSKEW
                if g >= 0:
                    ta, ca = divmod(g, NCH)
                    emit_av(ta, ca)
                # stream next tile's natural pass, one unit per chunk; the
                # max write into qaug row 64 rides a DMA whose completion
                # semaphore orders it against the fine matmuls of tile t+1
                if t + 1 < NT:
                    bl, sh = divmod(c, S // 512)
                    if bl < 4:
                        nat_unit((t + 1) * 4 + bl, sh)
                    if c % (S // 512) == 0 and 1 <= bl <= 3:
                        nat_finish((t + 1) * 4 + bl - 1)
                    if c == NCH - 1:
                        nat_finish((t + 1) * 4 + 3)
                # v projection streams through tile 0 at the DMA cadence
                if t == 0 and c >= 3 and (c - 3) % 3 == 0:
                    j = (c - 3) // 3
                    if j < NT:
                        v_mms(lv.pop(j), j)
                        if j + 2 < NT:
                            lv[j + 2] = proj_load(vt, j + 2, fp16)
                # epilogue of tile t-1 (avs for t-1 finish at c==3); both
                # normalizes emit at c==3 so all psA readers precede the
                # pav ring reallocation at c==4
                if t > 0:
                    if c == 3:
                        emit_normalize(t - 1, 0)
                        emit_normalize(t - 1, 1)
                    elif 6 <= c < 10:
                        emit_wo(t - 1, c - 6)
                # q projection for tile t+2: DMA early, matmuls later
                if t + 2 < NT:
                    if c == (27 if t == 0 else 2):
                        qp_pending[t + 2] = proj_load(qt, t + 2)
                    elif c == (30 if t == 0 else 16):
                        proj_mms(qp_pending.pop(t + 2), wq_sb, t + 2,
                                 qaug[0], qaug[1], q16)

        # drain: remaining avs, then last tile's epilogue
        for g in range(NT * NCH - AVSKEW, NT * NCH):
            ta, ca = divmod(g, NCH)
            emit_av(ta, ca)
        emit_normalize(NT - 1, 0)
        emit_normalize(NT - 1, 1)
        for b in range(4):
            emit_wo(NT - 1, b)

        if dbg is not None:
            nc.sync.dma_start(dbg["d_qaug0"][:], qaug[0][:].bitcast(f32))
            nc.sync.dma_start(dbg["d_qaug1"][:], qaug[1][:].bitcast(f32))
            nc.sync.dma_start(dbg["d_concat"][:], concat[:].bitcast(f32))
            nc.sync.dma_start(dbg["d_v"][:], v_sb[:].rearrange("p a b c -> p (a b c)"))

    nc.compile()
    return nc


def _prep_inputs(Q, K, V, W_Q, W_K, W_V, W_O):
    Q = np.ascontiguousarray(np.asarray(Q, dtype=np.float32))
    K = np.ascontiguousarray(np.asarray(K, dtype=np.float32))
    V = np.ascontiguousarray(np.asarray(V, dtype=np.float32))
    W_Q = np.asarray(W_Q, dtype=np.float32)
    W_K = np.asarray(W_K, dtype=np.float32)
    W_V = np.asarray(W_V, dtype=np.float32)
    W_O = np.asarray(W_O, dtype=np.float32)

    QT = np.ascontiguousarray(Q.T)
    KT = np.ascontiguousarray(K.T)
    VT = np.ascontiguousarray(V.T.astype(np.float16))
    scale = np.float32(0.125)  # 1/sqrt(64), exact power of two

    in_maps = []
    for c in range(NCORES):
        hA, hB = 2 * c, 2 * c + 1
        in_maps.append({
            "qt": QT,
            "kt": KT,
            "vt": VT,
            "wq": np.ascontiguousarray(np.concatenate([W_Q[hA], W_Q[hB]], axis=1)),
            "wk": np.ascontiguousarray(
                np.concatenate([W_K[hA] * scale, W_K[hB] * scale], axis=1)),
            "wv": np.ascontiguousarray(
                np.concatenate([W_V[hA], W_V[hB]], axis=1).astype(np.float16)),
            "wo": np.ascontiguousarray(
                W_O[c * 128:(c + 1) * 128, :].astype(np.float16)),
        })
    return in_maps


def kernel(Q, K, V, W_Q, W_K, W_V, W_O):
    global LAST_RESULT
    from concourse.bass_utils import run_bass_kernel_spmd

    S = np.asarray(Q).shape[0]
    nc = _cache.get(S)
    if nc is None:
        nc = _build(S)
        _cache[S] = nc

    in_maps = _prep_inputs(Q, K, V, W_Q, W_K, W_V, W_O)
    res = run_bass_kernel_spmd(nc, in_maps, list(range(NCORES)))
    LAST_RESULT = res
    parts = np.stack([res.results[i]["out"] for i in range(NCORES)])
    return parts.sum(axis=0, dtype=np.float32)
